# revision 1
# baseline (speedup 1.0000x reference)
"""Weighted L1 loss kernel for Trainium2 (8 NeuronCores, data-parallel).

reference:
    per_sample_l1 = mean(|out - target|, axis=1)   # [B], D=16
    weight        = 1 + 0.1 * x[:, 3]              # [B]
    result        = mean(per_sample_l1 * weight)   # scalar

Sharding: batch dim split across 8 cores (padded with zeros so each core
gets 128*980 samples). Each core computes a scalar partial sum of
sum_d |out-target| * (1 + 0.1*x[:,3]); the host sums the 8 partials and
divides by D*B.

Per-core pipeline, per tile of 128*K samples (K ramps 61->245->62 so
the first subtract starts as soon as ~1MB has landed and the final
tiles' compute tail is short; steady state is DMA-bound at ~358 GB/s;
a CCE accum-DMA subtract was tried and is both wrong and ~4x slower
than line rate on this hardware):
  sync  : DMA out/target tiles [128, K*16] and weight tile [128, K]
  gpsimd: d = out - target                  (tensor_tensor subtract)
          w' = 1 + 0.1*w                    (tensor_scalar)
  vector: l1[p,k] = sum_d |d[p,k,d]|        (tensor_reduce, abs)
          acc[p] += sum_k l1[p,k]*w'[p,k]   (mult + reduce + add;
          tensor_tensor_reduce would fuse these but crashes the exec
          unit on this hardware: NRT_EXEC_UNIT_UNRECOVERABLE)
Final: PE matmul ones.T @ acc -> PSUM [1,1] (a [128,1] SBUF->DRAM DMA
costs ~9us in 4B-per-partition descriptors; the matmul route is ~3us),
copy to SBUF via DVE, DMA one scalar out.
"""

import numpy as np

import concourse.tile as tile
from concourse import bacc, mybir
from concourse.bass_utils import run_bass_kernel_spmd

B = 1_000_000
D = 16
N_CORES = 8
P = 128            # SBUF partitions
K_LIST = [61, 122, 245, 245, 184, 61, 62]  # samples/partition per tile
KSUM = sum(K_LIST)                       # 980
BP = P * KSUM                            # 125_440 samples per core
BPAD = BP * N_CORES                      # 1_003_520

F32 = mybir.dt.float32

# Exposed for test harnesses: set TRACE=True before calling kernel() to get
# an NTFF profile; the BassKernelResults lands in LAST_RESULT.
TRACE = False
LAST_RESULT = None

_CACHE = {}


def _build():
    if "nc" in _CACHE:
        return _CACHE["nc"]

    nc = bacc.Bacc("TRN2", target_bir_lowering=False, debug=False,
                   num_devices=N_CORES)
    out_d = nc.dram_tensor("o", [BP, D], F32, kind="ExternalInput").ap()
    tgt_d = nc.dram_tensor("t", [BP, D], F32, kind="ExternalInput").ap()
    w_d = nc.dram_tensor("w", [BP], F32, kind="ExternalInput").ap()
    part_d = nc.dram_tensor("partial", [1, 1], F32, kind="ExternalOutput").ap()

    of = out_d.rearrange("s d -> (s d)")
    tf = tgt_d.rearrange("s d -> (s d)")

    with tile.TileContext(nc) as tc:
        with tc.tile_pool(name="io", bufs=4) as io_pool, \
             tc.tile_pool(name="dif", bufs=2) as dif_pool, \
             tc.tile_pool(name="small", bufs=4) as small_pool, \
             tc.tile_pool(name="acc", bufs=2) as acc_pool, \
             tc.tile_pool(name="fin", bufs=1) as fin_pool, \
             tc.tile_pool(name="ps", bufs=1, space="PSUM") as ps_pool:
            ones_t = fin_pool.tile([P, 1], F32, tag="ones")
            nc.gpsimd.memset(ones_t[:], 1.0)
            # one partial column per tile: independent writes, no chained
            # accumulator adds serializing DVE (they stalled DMA slot reuse)
            acc_all = fin_pool.tile([P, len(K_LIST)], F32, tag="acc_all")

            base = 0  # running sample offset
            for ti, K in enumerate(K_LIST):
                FW = K * D
                # samples [base, base+128*K): partition p holds samples
                # base + p*K .. base + p*K + K-1, each 16 contiguous floats
                ov = of[base * D:(base + P * K) * D].rearrange(
                    "(p f) -> p f", p=P)
                tv = tf[base * D:(base + P * K) * D].rearrange(
                    "(p f) -> p f", p=P)
                wv = w_d[base:base + P * K].rearrange("(p k) -> p k", p=P)

                o_t = io_pool.tile([P, FW], F32, tag="o")
                nc.sync.dma_start(o_t[:], ov)
                g_t = io_pool.tile([P, FW], F32, tag="g")
                nc.sync.dma_start(g_t[:], tv)
                w_t = small_pool.tile([P, K], F32, tag="w")
                nc.sync.dma_start(w_t[:], wv)

                wp_t = small_pool.tile([P, K], F32, tag="wp")
                nc.gpsimd.tensor_scalar(wp_t[:], w_t[:], 0.1, 1.0,
                                        mybir.AluOpType.mult,
                                        mybir.AluOpType.add)

                d_t = dif_pool.tile([P, FW], F32, tag="d")
                # DVE has ~1.7us/tile of slack after its reduce chain;
                # give it a 25% sample-aligned slice of the subtract
                sp = ((K * 75 + 99) // 100) * D
                nc.gpsimd.tensor_tensor(d_t[:, :sp], o_t[:, :sp],
                                        g_t[:, :sp],
                                        mybir.AluOpType.subtract)
                nc.vector.tensor_tensor(d_t[:, sp:], o_t[:, sp:],
                                        g_t[:, sp:],
                                        mybir.AluOpType.subtract)

                l1_t = small_pool.tile([P, K], F32, tag="l1")
                nc.vector.tensor_reduce(
                    l1_t[:],
                    d_t[:].rearrange("p (k d) -> p k d", d=D),
                    axis=mybir.AxisListType.X,
                    op=mybir.AluOpType.add,
                    apply_absolute_value=True,
                )

                prod_t = small_pool.tile([P, K], F32, tag="prod")
                nc.vector.tensor_tensor(prod_t[:], l1_t[:], wp_t[:],
                                        mybir.AluOpType.mult)
                nc.vector.tensor_reduce(acc_all[:, ti:ti + 1], prod_t[:],
                                        axis=mybir.AxisListType.X,
                                        op=mybir.AluOpType.add)
                base += P * K

            accf_t = acc_pool.tile([P, 1], F32, tag="accf")
            nc.vector.tensor_reduce(accf_t[:], acc_all[:],
                                    axis=mybir.AxisListType.X,
                                    op=mybir.AluOpType.add)
            psum_t = ps_pool.tile([1, 1], F32, tag="ps")
            nc.tensor.matmul(psum_t[:], accf_t[:], ones_t[:],
                             start=True, stop=True)
            fin_t = fin_pool.tile([1, 1], F32, tag="fin")
            nc.vector.tensor_copy(fin_t[:], psum_t[:])
            nc.sync.dma_start(part_d[:], fin_t[:])

    nc.compile()
    _CACHE["nc"] = nc
    return nc


def kernel(out, target, x):
    global LAST_RESULT
    nc = _build()

    out = np.ascontiguousarray(out, dtype=np.float32)
    target = np.ascontiguousarray(target, dtype=np.float32)
    w = np.ascontiguousarray(np.asarray(x, dtype=np.float32)[:, 3])

    o_p = np.zeros((BPAD, D), np.float32)
    o_p[:B] = out
    t_p = np.zeros((BPAD, D), np.float32)
    t_p[:B] = target
    w_p = np.zeros(BPAD, np.float32)
    w_p[:B] = w

    in_maps = []
    for c in range(N_CORES):
        sl = slice(c * BP, (c + 1) * BP)
        in_maps.append({"o": o_p[sl], "t": t_p[sl], "w": w_p[sl]})

    res = run_bass_kernel_spmd(nc, in_maps, list(range(N_CORES)), trace=TRACE)
    LAST_RESULT = res

    total = np.float64(0.0)
    for r in res.results:
        total += np.float64(r["partial"][0, 0])
    return np.array(total / (D * B), dtype=np.float32)



# revision 4
# speedup vs baseline: 1.3280x; 1.3280x over previous
"""Weighted L1 loss kernel for Trainium2 (8 NeuronCores, data-parallel).

reference:
    per_sample_l1 = mean(|out - target|, axis=1)   # [B], D=16
    weight        = 1 + 0.1 * x[:, 3]              # [B]
    result        = mean(per_sample_l1 * weight)   # scalar

Host side: inputs are cast to bf16 (rel tolerance is 2e-2; bf16 end-to-end
error is ~2e-4) and re-laid out per core into [128, 16*KSUM] tile-contiguous
d-major blocks, so each on-device tile [128, 16*K] holds 16 feature planes
of K samples back to back. This halves HBM traffic vs f32 (8.3MB/core,
~23us at 358 GB/s) which is the roofline for this kernel.

Device per tile (d-major layout makes the D-reduction a binary tree of
contiguous tensor_tensor adds, which run at 2x DVE mode in bf16 --
tensor_reduce only has a 1x uop and would cost 16.3us/core alone):
  sync  : DMA o/g tiles [128, 16K] bf16 (w for the whole core is one DMA)
  gpsimd: wp = 1 + 0.1*w       (tiny) ; leading SP_FRAC of the subtract
  scalar: a = Abs(d)           (ACT engine, otherwise idle)
  vector: rest of subtract (bf16 TT, 2x) ;
          tree: 8K+4K+2K+K adds (2x) -> l1 [128, K] ;
          scalar_tensor_tensor: prod = l1*wp, accum_out = acc column (f32)
Final: reduce acc columns, PE matmul ones.T @ acc -> PSUM [1,1], copy to
SBUF, DMA one f32 scalar out per core; host sums 8 partials / (D*B).
"""

import numpy as np
import ml_dtypes

import concourse.tile as tile
from concourse import bacc, mybir
from concourse.bass_utils import run_bass_kernel_spmd

B = 1_000_000
D = 16
N_CORES = 8
P = 128                                  # SBUF partitions
K_LIST = [64, 128, 256, 256, 128, 84, 64]  # samples/partition per tile
KSUM = sum(K_LIST)                       # 980
BP = P * KSUM                            # 125_440 samples per core
BPAD = BP * N_CORES                      # 1_003_520
FTOT = D * KSUM                          # bf16 elems per partition per tensor

SP_NUM, SP_DEN = 2, 5                    # gpsimd share of the subtract

F32 = mybir.dt.float32
BF16 = mybir.dt.bfloat16
NP_BF16 = ml_dtypes.bfloat16

# Exposed for test harnesses: set TRACE=True before calling kernel() to get
# an NTFF profile; the BassKernelResults lands in LAST_RESULT.
TRACE = False
LAST_RESULT = None

_CACHE = {}


def _build():
    if "nc" in _CACHE:
        return _CACHE["nc"]

    nc = bacc.Bacc("TRN2", target_bir_lowering=False, debug=False,
                   num_devices=N_CORES)
    o_d = nc.dram_tensor("o", [P, FTOT], BF16, kind="ExternalInput").ap()
    t_d = nc.dram_tensor("t", [P, FTOT], BF16, kind="ExternalInput").ap()
    w_d = nc.dram_tensor("w", [P, KSUM], BF16, kind="ExternalInput").ap()
    part_d = nc.dram_tensor("partial", [1, 1], F32, kind="ExternalOutput").ap()

    T = len(K_LIST)

    with tile.TileContext(nc) as tc:
        with tc.tile_pool(name="io", bufs=3) as io_pool, \
             tc.tile_pool(name="dif", bufs=2) as dif_pool, \
             tc.tile_pool(name="small", bufs=4) as small_pool, \
             tc.tile_pool(name="fin", bufs=1) as fin_pool, \
             tc.tile_pool(name="ps", bufs=1, space="PSUM") as ps_pool:
            ones_t = fin_pool.tile([P, 1], F32, tag="ones")
            nc.gpsimd.memset(ones_t[:], 1.0)
            # whole-core weight row: one DMA up front
            w_all = fin_pool.tile([P, KSUM], BF16, tag="w_all")
            nc.sync.dma_start(w_all[:], w_d)
            # one f32 partial column per tile (independent writes)
            acc_all = fin_pool.tile([P, T], F32, tag="acc_all")

            col = 0
            kbase = 0
            for ti, K in enumerate(K_LIST):
                FW = D * K
                o_t = io_pool.tile([P, FW], BF16, tag="o")
                nc.sync.dma_start(o_t[:], o_d[:, col:col + FW])
                g_t = io_pool.tile([P, FW], BF16, tag="g")
                nc.sync.dma_start(g_t[:], t_d[:, col:col + FW])

                wp_t = small_pool.tile([P, K], BF16, tag="wp")
                nc.gpsimd.tensor_scalar(wp_t[:], w_all[:, kbase:kbase + K],
                                        0.1, 1.0,
                                        mybir.AluOpType.mult,
                                        mybir.AluOpType.add)

                d_t = dif_pool.tile([P, FW], BF16, tag="d")
                sp = (FW * SP_NUM // SP_DEN) & ~1  # even -> 4B aligned
                nc.gpsimd.tensor_tensor(d_t[:, :sp], o_t[:, :sp],
                                        g_t[:, :sp],
                                        mybir.AluOpType.subtract)
                nc.vector.tensor_tensor(d_t[:, sp:], o_t[:, sp:],
                                        g_t[:, sp:],
                                        mybir.AluOpType.subtract)

                # |d| on the ACT engine (abs_max is invalid ISA on DVE
                # tensor_scalar; ACT is idle anyway)
                a_t = dif_pool.tile([P, FW], BF16, tag="a")
                nc.scalar.activation(a_t[:], d_t[:],
                                     mybir.ActivationFunctionType.Abs)

                # binary tree over the 16 feature planes (each [P, K])
                h = FW // 2
                while h >= 2 * K:
                    nc.vector.tensor_tensor(a_t[:, :h], a_t[:, :h],
                                            a_t[:, h:2 * h],
                                            mybir.AluOpType.add)
                    h //= 2
                l1_t = small_pool.tile([P, K], BF16, tag="l1")
                nc.vector.tensor_tensor(l1_t[:], a_t[:, :K], a_t[:, K:2 * K],
                                        mybir.AluOpType.add)

                # prod = l1 * wp ; acc column = per-partition sum (f32)
                prod_t = small_pool.tile([P, K], BF16, tag="prod")
                nc.vector.scalar_tensor_tensor(
                    prod_t[:], l1_t[:], 1.0, wp_t[:],
                    mybir.AluOpType.bypass, mybir.AluOpType.mult,
                    accum_out=acc_all[:, ti:ti + 1])
                col += FW
                kbase += K

            accf_t = fin_pool.tile([P, 1], F32, tag="accf")
            nc.vector.tensor_reduce(accf_t[:], acc_all[:],
                                    axis=mybir.AxisListType.X,
                                    op=mybir.AluOpType.add)
            psum_t = ps_pool.tile([1, 1], F32, tag="ps")
            nc.tensor.matmul(psum_t[:], accf_t[:], ones_t[:],
                             start=True, stop=True)
            fin_t = fin_pool.tile([1, 1], F32, tag="fin")
            nc.vector.tensor_copy(fin_t[:], psum_t[:])
            nc.sync.dma_start(part_d[:], fin_t[:])

    nc.compile()
    _CACHE["nc"] = nc
    return nc


def _host_prep(out, target, x):
    """Cast to bf16 and lay out per core as [128, 16*KSUM] with
    tile-contiguous d-major blocks: columns [16*k0, 16*(k0+K)) of tile
    (k0, K) hold planes d=0..15 of samples k0..k0+K-1."""
    w = np.asarray(x, dtype=np.float32)[:, 3]

    o_p = np.zeros((BPAD, D), NP_BF16)
    o_p[:B] = np.asarray(out, dtype=np.float32).astype(NP_BF16)
    t_p = np.zeros((BPAD, D), NP_BF16)
    t_p[:B] = np.asarray(target, dtype=np.float32).astype(NP_BF16)
    w_p = np.zeros(BPAD, NP_BF16)
    w_p[:B] = w.astype(NP_BF16)

    in_maps = []
    for c in range(N_CORES):
        sl = slice(c * BP, (c + 1) * BP)
        oc = o_p[sl].reshape(P, KSUM, D)
        tc_ = t_p[sl].reshape(P, KSUM, D)
        o_dev = np.empty((P, FTOT), NP_BF16)
        t_dev = np.empty((P, FTOT), NP_BF16)
        k0 = 0
        for K in K_LIST:
            blk = slice(D * k0, D * (k0 + K))
            o_dev[:, blk] = oc[:, k0:k0 + K, :].transpose(0, 2, 1).reshape(P, D * K)
            t_dev[:, blk] = tc_[:, k0:k0 + K, :].transpose(0, 2, 1).reshape(P, D * K)
            k0 += K
        w_dev = np.ascontiguousarray(w_p[sl].reshape(P, KSUM))
        in_maps.append({"o": o_dev, "t": t_dev, "w": w_dev})
    return in_maps


def kernel(out, target, x):
    global LAST_RESULT
    nc = _build()
    in_maps = _host_prep(out, target, x)

    res = run_bass_kernel_spmd(nc, in_maps, list(range(N_CORES)), trace=TRACE)
    LAST_RESULT = res

    total = np.float64(0.0)
    for r in res.results:
        total += np.float64(r["partial"][0, 0])
    return np.array(total / (D * B), dtype=np.float32)


# revision 5
# speedup vs baseline: 1.4379x; 1.0828x over previous
"""Weighted L1 loss kernel for Trainium2 (8 NeuronCores, data-parallel).

reference:
    per_sample_l1 = mean(|out - target|, axis=1)   # [B], D=16
    weight        = 1 + 0.1 * x[:, 3]              # [B]
    result        = mean(per_sample_l1 * weight)   # scalar

Host side: inputs are cast to bf16 (rel tolerance is 2e-2; bf16 end-to-end
error is ~2e-4) and re-laid out per core into [128, 16*KSUM] tile-contiguous
d-major blocks, so each on-device tile [128, 16*K] holds 16 feature planes
of K samples back to back. This halves HBM traffic vs f32 (8.3MB/core,
~23us at 358 GB/s) which is the roofline for this kernel.

Device per tile, engine-balanced so every engine stays under the DMA rate
(measured bf16 rates, ns/elem/lane: DVE TT 0.54, DVE tensor_scalar 0.28,
DVE stt 1.06, ACT abs 0.87, GpSimd TT-sub 2.42):
  sync  : DMA o/g tiles [128, 16K] bf16 (w for the whole core: one DMA)
  gpsimd: leading 30% of subtract; wp = 1 + 0.1*w (tiny)
  scalar: |d| on the leading 85% of columns (ACT Abs, table primed early)
  vector: trailing 70% of subtract (TT 2x);
          |d| on trailing 15% via bitwise AND 0x7FFF on a u16 view (4x);
          D-reduce as binary tree of in-place TT adds 8K+4K+2K (2x), last
          level into l1 [128, K]  (tensor_reduce has only a 1x uop -- the
          tree at 2x is ~2x faster);
          scalar_tensor_tensor: prod = l1*wp, accum_out = acc column (f32)
Final: reduce acc columns, PE matmul ones.T @ acc -> PSUM [1,1], copy to
SBUF, DMA one f32 scalar out per core; host sums 8 partials / (D*B).
"""

import numpy as np
import ml_dtypes

import concourse.tile as tile
from concourse import bacc, mybir
from concourse.bass_utils import run_bass_kernel_spmd

B = 1_000_000
D = 16
N_CORES = 8
P = 128                                  # SBUF partitions
K_LIST = [64, 128, 256, 256, 128, 84, 64]  # samples/partition per tile
KSUM = sum(K_LIST)                       # 980
BP = P * KSUM                            # 125_440 samples per core
BPAD = BP * N_CORES                      # 1_003_520
FTOT = D * KSUM                          # bf16 elems per partition per tensor

SUB_GP_NUM, SUB_GP_DEN = 3, 10           # gpsimd share of the subtract
ABS_ACT_NUM, ABS_ACT_DEN = 17, 20        # ACT share of the abs

F32 = mybir.dt.float32
BF16 = mybir.dt.bfloat16
U16 = mybir.dt.uint16
NP_BF16 = ml_dtypes.bfloat16

TRACE = False
LAST_RESULT = None

_CACHE = {}


def _build():
    if "nc" in _CACHE:
        return _CACHE["nc"]

    nc = bacc.Bacc("TRN2", target_bir_lowering=False, debug=False,
                   num_devices=N_CORES)
    o_d = nc.dram_tensor("o", [P, FTOT], BF16, kind="ExternalInput").ap()
    t_d = nc.dram_tensor("t", [P, FTOT], BF16, kind="ExternalInput").ap()
    w_d = nc.dram_tensor("w", [P, KSUM], BF16, kind="ExternalInput").ap()
    part_d = nc.dram_tensor("partial", [1, 1], F32, kind="ExternalOutput").ap()

    T = len(K_LIST)

    with tile.TileContext(nc) as tc:
        with tc.tile_pool(name="io", bufs=5) as io_pool, \
             tc.tile_pool(name="dif", bufs=3) as dif_pool, \
             tc.tile_pool(name="small", bufs=4) as small_pool, \
             tc.tile_pool(name="fin", bufs=1) as fin_pool, \
             tc.tile_pool(name="ps", bufs=1, space="PSUM") as ps_pool:
            ones_t = fin_pool.tile([P, 2], F32, tag="ones")
            nc.gpsimd.memset(ones_t[:], 1.0)
            # prime the ACT function table while the first DMAs run
            prime_t = fin_pool.tile([P, 2], F32, tag="prime")
            nc.scalar.activation(prime_t[:], ones_t[:],
                                 mybir.ActivationFunctionType.Abs)
            # whole-core weight row: one DMA up front
            w_all = fin_pool.tile([P, KSUM], BF16, tag="w_all")
            nc.sync.dma_start(w_all[:], w_d)
            # one f32 partial column per tile (independent writes)
            acc_all = fin_pool.tile([P, T], F32, tag="acc_all")

            col = 0
            kbase = 0
            for ti, K in enumerate(K_LIST):
                FW = D * K
                o_t = io_pool.tile([P, FW], BF16, tag="o")
                nc.sync.dma_start(o_t[:], o_d[:, col:col + FW])
                g_t = io_pool.tile([P, FW], BF16, tag="g")
                nc.sync.dma_start(g_t[:], t_d[:, col:col + FW])

                wp_t = small_pool.tile([P, K], BF16, tag="wp")
                nc.gpsimd.tensor_scalar(wp_t[:], w_all[:, kbase:kbase + K],
                                        0.1, 1.0,
                                        mybir.AluOpType.mult,
                                        mybir.AluOpType.add)

                d_t = dif_pool.tile([P, FW], BF16, tag="d")
                sp = (FW * SUB_GP_NUM // SUB_GP_DEN) & ~31
                nc.gpsimd.tensor_tensor(d_t[:, :sp], o_t[:, :sp],
                                        g_t[:, :sp],
                                        mybir.AluOpType.subtract)
                nc.vector.tensor_tensor(d_t[:, sp:], o_t[:, sp:],
                                        g_t[:, sp:],
                                        mybir.AluOpType.subtract)

                a_t = dif_pool.tile([P, FW], BF16, tag="a")
                ca = (FW * ABS_ACT_NUM // ABS_ACT_DEN) & ~31
                nc.scalar.activation(a_t[:, :ca], d_t[:, :ca],
                                     mybir.ActivationFunctionType.Abs)
                nc.vector.tensor_scalar(a_t[:, ca:].bitcast(U16),
                                        d_t[:, ca:].bitcast(U16),
                                        0x7FFF, None,
                                        mybir.AluOpType.bitwise_and)

                # binary tree over the 16 feature planes (each [P, K])
                h = FW // 2
                while h >= 2 * K:
                    nc.vector.tensor_tensor(a_t[:, :h], a_t[:, :h],
                                            a_t[:, h:2 * h],
                                            mybir.AluOpType.add)
                    h //= 2
                l1_t = small_pool.tile([P, K], BF16, tag="l1")
                nc.vector.tensor_tensor(l1_t[:], a_t[:, :K], a_t[:, K:2 * K],
                                        mybir.AluOpType.add)

                # prod = l1 * wp ; acc column = per-partition sum (f32)
                prod_t = small_pool.tile([P, K], BF16, tag="prod")
                nc.vector.scalar_tensor_tensor(
                    prod_t[:], l1_t[:], 1.0, wp_t[:],
                    mybir.AluOpType.bypass, mybir.AluOpType.mult,
                    accum_out=acc_all[:, ti:ti + 1])
                col += FW
                kbase += K

            accf_t = fin_pool.tile([P, 1], F32, tag="accf")
            nc.vector.tensor_reduce(accf_t[:], acc_all[:],
                                    axis=mybir.AxisListType.X,
                                    op=mybir.AluOpType.add)
            psum_t = ps_pool.tile([1, 1], F32, tag="ps")
            nc.tensor.matmul(psum_t[:], accf_t[:], ones_t[:, :1],
                             start=True, stop=True)
            fin_t = fin_pool.tile([1, 1], F32, tag="fin")
            nc.vector.tensor_copy(fin_t[:], psum_t[:])
            nc.sync.dma_start(part_d[:], fin_t[:])

    nc.compile()
    _CACHE["nc"] = nc
    return nc


def _host_prep(out, target, x):
    """Cast to bf16 and lay out per core as [128, 16*KSUM] with
    tile-contiguous d-major blocks: columns [16*k0, 16*(k0+K)) of tile
    (k0, K) hold planes d=0..15 of samples k0..k0+K-1."""
    w = np.asarray(x, dtype=np.float32)[:, 3]

    o_p = np.zeros((BPAD, D), NP_BF16)
    o_p[:B] = np.asarray(out, dtype=np.float32).astype(NP_BF16)
    t_p = np.zeros((BPAD, D), NP_BF16)
    t_p[:B] = np.asarray(target, dtype=np.float32).astype(NP_BF16)
    w_p = np.zeros(BPAD, NP_BF16)
    w_p[:B] = w.astype(NP_BF16)

    in_maps = []
    for c in range(N_CORES):
        sl = slice(c * BP, (c + 1) * BP)
        oc = o_p[sl].reshape(P, KSUM, D)
        tc_ = t_p[sl].reshape(P, KSUM, D)
        o_dev = np.empty((P, FTOT), NP_BF16)
        t_dev = np.empty((P, FTOT), NP_BF16)
        k0 = 0
        for K in K_LIST:
            blk = slice(D * k0, D * (k0 + K))
            o_dev[:, blk] = oc[:, k0:k0 + K, :].transpose(0, 2, 1).reshape(P, D * K)
            t_dev[:, blk] = tc_[:, k0:k0 + K, :].transpose(0, 2, 1).reshape(P, D * K)
            k0 += K
        w_dev = np.ascontiguousarray(w_p[sl].reshape(P, KSUM))
        in_maps.append({"o": o_dev, "t": t_dev, "w": w_dev})
    return in_maps


def kernel(out, target, x):
    global LAST_RESULT
    nc = _build()
    in_maps = _host_prep(out, target, x)

    res = run_bass_kernel_spmd(nc, in_maps, list(range(N_CORES)), trace=TRACE)
    LAST_RESULT = res

    total = np.float64(0.0)
    for r in res.results:
        total += np.float64(r["partial"][0, 0])
    return np.array(total / (D * B), dtype=np.float32)
